# revision 33
# baseline (speedup 1.0000x reference)
"""Multi-head differential attention (full-width) on 8 Trainium2 NeuronCores.

Contract: kernel(**inputs) takes the FULL inputs of reference.setup_inputs()
and returns the FULL [8, 512, 8192] float32 output.

Strategy: pure data-parallel over batch — one batch element per NeuronCore.

Algebraic reformulation (host precompute, device GEMMs):
  scores_h = q_h k_hᵀ·scale = x M_h xᵀ (+ rank-1 bias row),
      M_h = Wq_hᵀ Wk_h · scale   — ONE [512,4096]x[4096,4096] GEMM per half
      (y_h = x M_h) instead of separate q and k projections.
  out = P V = (P x) Wvᵀ + rowsum(P)·bvᵀ — contracting P into x FIRST replaces
      the [512,512]x[512,8192] attention-V GEMM with a [512,512]x[512,4096]
      one; the rank-1 bv term rides for free in the PSUM-drain op (rowsum(P)
      falls out of the combine's accum_out).

All GEMMs run in bf16 (validated: rel_inf ≈ 7e-3 vs the 2e-2 gate), PSUM
accumulation is fp32. Per core the fused Tile kernel computes:
  - yT blocks o-block by o-block with the score matmuls fused into the same
    loop (only a rotating 2-tile window of yT is ever live),
  - softmax via ScalarE Exp with accum_out row-sums (scores bounded ~±25, so
    no max-subtraction is needed in fp32),
  - differential combine P = e1/d1 - lam ⊙ e2/d2 on VectorE (bf16 out),
  - PE-transpose of P, Zᵀ = xᵀ Pᵀ, then a streamed Wv GEMM that writes each
    512-column output tile as it completes.

All inputs are repacked host-side (layout permutation + dtype cast) so every
DMA lands with long contiguous per-partition runs.
"""
import ml_dtypes
import numpy as np
from contextlib import ExitStack

import concourse.bass as bass
import concourse.mybir as mybir
import concourse.tile as tile
from concourse.bass_utils import run_bass_kernel_spmd
from concourse.masks import make_identity

F32 = mybir.dt.float32
F32R = mybir.dt.float32r
BF16 = mybir.dt.bfloat16
P = 128
B = 8
S = 512          # sequence length (= d_head for the lambda broadcast)
DM = 4096        # model dim (projection contraction)
DH = 4096        # width of each q/k half (attention contraction)
D2 = 2 * DH      # projection output width
NQB = S // P     # 4 query blocks
NJB = S // P     # 4 key blocks
NT = DM // P     # 32 contraction tiles
NOB = D2 // P    # 64 y o-blocks (32 per half)
HOB = DH // P    # 32 o-blocks per half
NODT = D2 // 512  # 16 output column tiles
WV_CH = 8        # c-tiles per streamed Wv chunk (4 chunks of 8 = 32)
SCALE = float(1.0 / np.sqrt(512.0))

MAX_WAITS = 1  # this walrus build allows one sync-wait per instruction


def _split_sync_waits(nc):
    """Move excess per-instruction sync waits onto preceding no-ops (same
    engine, program order) — semantically identical, keeps walrus happy."""
    for f in nc.m.functions:
        for bb in f.blocks:
            new_insts = []
            for inst in bb.instructions:
                si = inst.sync_info
                if si is not None and si.on_wait and len(si.on_wait) > MAX_WAITS:
                    waits = list(si.on_wait)
                    excess, keep = waits[MAX_WAITS:], waits[:MAX_WAITS]
                    for ci in range(0, len(excess), MAX_WAITS):
                        new_insts.append(mybir.InstNoOp(
                            name=f"{inst.name}-waitsplit{ci}",
                            engine=inst.engine, ins=[], outs=[],
                            sync_info=mybir.SyncInfo(
                                on_wait=excess[ci:ci + MAX_WAITS], on_update=[]),
                            text_hint="waitsplit"))
                    si.on_wait = keep
                new_insts.append(inst)
            bb.instructions = new_insts


def build_nc():
    nc = bass.Bass()
    xtb = nc.declare_dram_parameter("xtb", [P, NT, S], BF16, isOutput=False)
    xre = nc.declare_dram_parameter("xre", [P, NJB, DM], BF16, isOutput=False)
    wm = nc.declare_dram_parameter("wm", [NOB, P, NT, P], BF16, isOutput=False)
    wve = nc.declare_dram_parameter("wve", [NODT, P, NT, 512], BF16, isOutput=False)
    whs = nc.declare_dram_parameter("whs", [2, S], F32R, isOutput=False)
    lamb = nc.declare_dram_parameter("lamb", [P, S], BF16, isOutput=False)
    bvb = nc.declare_dram_parameter("bvb", [P, NODT, 512], BF16, isOutput=False)
    ones = nc.declare_dram_parameter("ones", [P], F32R, isOutput=False)
    out = nc.declare_dram_parameter("out", [S, D2], F32, isOutput=True)

    with tile.TileContext(nc) as tc, ExitStack() as top:
        const = top.enter_context(tc.tile_pool(name="const", bufs=1))
        ones_row = const.tile([1, P], F32R, name="ones_row")
        nc.sync.dma_start(ones_row[:], ones[None, :])
        whs_sb = const.tile([1, 2, S], F32R, name="whs_sb")
        nc.sync.dma_start(whs_sb[:], whs.rearrange("(o h) s -> o h s", o=1))
        ident = const.tile([P, P], BF16, name="ident")
        make_identity(nc, ident[:])
        # x^T blocks feed both phase-A matmul operands; 16 chunks spread the
        # startup-critical load across all DMA queues (finer splits measured
        # slower: per-transfer overhead without earlier arrival).
        xtb_sb = const.tile([P, NT, S], BF16, name="xtb_sb")
        for xc in range(16):
            nc.sync.dma_start(xtb_sb[:, xc * 2:(xc + 1) * 2, :],
                              xtb[:, xc * 2:(xc + 1) * 2, :])
        # phase-B-only tensors (DMAs issued mid-phase-A, see below)
        xre_sb = const.tile([P, NJB, DM], BF16, name="xre_sb")
        lamb_sb = const.tile([P, S], BF16, name="lamb_sb")
        bvb_sb = const.tile([P, NODT, 512], BF16, name="bvb_sb")

        e_sb = const.tile([P, 2, NQB, S], BF16, name="e_sb")
        d_sb = const.tile([P, 2, NQB], F32, name="d_sb")
        r_sb = const.tile([P, 2, NQB], F32, name="r_sb")
        rs_sb = const.tile([P, NQB], F32, name="rs_sb")
        P_sb = const.tile([P, NQB, S], BF16, name="P_sb")
        PT_sb = const.tile([P, NJB, S], BF16, name="PT_sb")
        ZT_sb = const.tile([P, NT, S], BF16, name="ZT_sb")

        # ---- Phase A: fused y = x@M_h blocks + score accumulation ----
        with ExitStack() as phA:
            w0p = phA.enter_context(tc.tile_pool(name="w0p", bufs=8))
            wqk = phA.enter_context(tc.tile_pool(name="wqk", bufs=6))
            qksb = phA.enter_context(tc.tile_pool(name="qksb", bufs=4))
            ps_proj = phA.enter_context(tc.tile_pool(name="ps_proj", bufs=3, space="PSUM"))
            ps_scores = phA.enter_context(tc.tile_pool(name="ps_scores", bufs=5, space="PSUM"))

            for h in range(2):
                sc_tiles = [ps_scores.tile([P, S], F32, name=f"sc_{h}_{qbk}", tag="sc")
                            for qbk in range(NQB)]
                prev = None

                def emit_scores(i, y_sb):
                    for qbk in range(NQB):
                        nc.tensor.matmul(sc_tiles[qbk][:],
                                         y_sb[:, qbk * P:(qbk + 1) * P],
                                         xtb_sb[:, i, :],
                                         start=False, stop=(i == HOB - 1))

                for i in range(HOB):
                    ob = h * HOB + i
                    if h == 1 and i == HOB // 2:
                        for jb in range(NJB):
                            nc.sync.dma_start(xre_sb[:, jb:jb + 1, :],
                                              xre[:, jb:jb + 1, :])
                        nc.sync.dma_start(lamb_sb[:], lamb[:])
                        nc.sync.dma_start(bvb_sb[:, :NODT // 2, :],
                                          bvb[:, :NODT // 2, :])
                        nc.sync.dma_start(bvb_sb[:, NODT // 2:, :],
                                          bvb[:, NODT // 2:, :])
                    pq = ps_proj.tile([P, S], F32, name="pq", tag="pp")
                    # the very first weight block rides in 4 small chunks so
                    # the PE can start while x^T is still streaming in
                    ncw = 4 if ob == 0 else 2
                    for cw in range(ncw):
                        nw = NT // ncw
                        pool = w0p if ob == 0 else wqk
                        wt = pool.tile([P, nw, P], BF16, name="wt_m", tag="w")
                        nc.sync.dma_start(wt[:], wm[ob][:, cw * nw:(cw + 1) * nw, :])
                        for tt in range(nw):
                            t = cw * nw + tt
                            nc.tensor.matmul(pq[:], wt[:, tt, :], xtb_sb[:, t, :],
                                             start=(t == 0), stop=(t == NT - 1))
                    y_sb = qksb.tile([P, S], BF16, name="y_sb", tag="qk")
                    nc.vector.tensor_copy(out=y_sb[:], in_=pq[:])
                    if i == 1:
                        # rank-1 q/k-bias correction row (zero for zero biases)
                        # seeds each accumulation group; deferred past y(0) so
                        # the h=1 seeds never wait on the h=0 exp reading the
                        # recycled PSUM bank.
                        for qbk in range(NQB):
                            nc.tensor.matmul(sc_tiles[qbk][:], ones_row[:],
                                             whs_sb[:, h, :], start=True, stop=False)
                    if prev is not None:
                        emit_scores(*prev)
                    prev = (i, y_sb)
                emit_scores(*prev)
                for qbk in range(NQB):
                    nc.scalar.activation(e_sb[:, h, qbk, :], sc_tiles[qbk][:],
                                         mybir.ActivationFunctionType.Exp,
                                         accum_out=d_sb[:, h, qbk:qbk + 1])
                nc.vector.reciprocal(r_sb[:, h, :], d_sb[:, h, :])

        # ---- Phase B: combine, P transpose, Z^T = x^T P^T, out = Z Wv^T ----
        with ExitStack() as phB1:
            cmb = phB1.enter_context(tc.tile_pool(name="cmb", bufs=2))
            ps_tr = phB1.enter_context(tc.tile_pool(name="ps_tr", bufs=2, space="PSUM"))
            ps_z = phB1.enter_context(tc.tile_pool(name="ps_z", bufs=3, space="PSUM"))

            for qbk in range(NQB):
                # P = (e1/d1) - lam ⊙ (e2/d2), two fused DVE ops; the second
                # also emits rowsum(P) for the v-bias rank-1 term.
                tmp = cmb.tile([P, S], BF16, name="tmp", tag="tmp")
                nc.vector.scalar_tensor_tensor(
                    tmp[:], e_sb[:, 1, qbk, :], r_sb[:, 1, qbk:qbk + 1], lamb_sb[:],
                    mybir.AluOpType.mult, mybir.AluOpType.mult)
                nc.vector.scalar_tensor_tensor(
                    P_sb[:, qbk, :], e_sb[:, 0, qbk, :], r_sb[:, 0, qbk:qbk + 1], tmp[:],
                    mybir.AluOpType.mult, mybir.AluOpType.subtract,
                    accum_out=rs_sb[:, qbk:qbk + 1])
            for kbk in range(NJB):
                for qbk in range(NQB):
                    pt2 = ps_tr.tile([P, P], BF16, name="pt2", tag="pt")
                    nc.tensor.transpose(pt2[:], P_sb[:, qbk, kbk * P:(kbk + 1) * P],
                                        ident[:])
                    nc.vector.tensor_copy(out=PT_sb[:, kbk, qbk * P:(qbk + 1) * P],
                                          in_=pt2[:])
            # Z^T[c, q] = sum_j x[j, c] P^T[j, q]
            for cb in range(NT):
                pz = ps_z.tile([P, S], F32, name="pz", tag="z")
                for jb in range(NJB):
                    nc.tensor.matmul(pz[:], xre_sb[:, jb, cb * P:(cb + 1) * P],
                                     PT_sb[:, jb, :],
                                     start=(jb == 0), stop=(jb == NJB - 1))
                nc.vector.tensor_copy(out=ZT_sb[:, cb, :], in_=pz[:])

        with ExitStack() as phB2:
            wvp = phB2.enter_context(tc.tile_pool(name="wvp", bufs=4))
            osb = phB2.enter_context(tc.tile_pool(name="osb", bufs=4))
            ps_vp = phB2.enter_context(tc.tile_pool(name="ps_vp", bufs=6, space="PSUM"))

            for odt in range(NODT):
                pv = [ps_vp.tile([P, 512], F32, name=f"pv{qb}", tag="vp")
                      for qb in range(NQB)]
                for c in range(NT // WV_CH):
                    wvt = wvp.tile([P, WV_CH, 512], BF16, name="wvt", tag="wv")
                    nc.sync.dma_start(wvt[:], wve[odt][:, c * WV_CH:(c + 1) * WV_CH, :])
                    for qb in range(NQB):
                        for tt in range(WV_CH):
                            t = c * WV_CH + tt
                            nc.tensor.matmul(pv[qb][:], ZT_sb[:, t, qb * P:(qb + 1) * P],
                                             wvt[:, tt, :],
                                             start=(t == 0), stop=(t == NT - 1))
                for qb in range(NQB):
                    # out tile = Z Wv^T (+ rowsum(P)·bv^T, free in the drain op)
                    o_st = osb.tile([P, 512], F32, name="o_st", tag="o")
                    nc.vector.scalar_tensor_tensor(
                        o_st[:], bvb_sb[:, odt, :], rs_sb[:, qb:qb + 1], pv[qb][:],
                        mybir.AluOpType.mult, mybir.AluOpType.add)
                    nc.sync.dma_start(
                        out[qb * P:(qb + 1) * P, odt * 512:(odt + 1) * 512],
                        o_st[:])

    _split_sync_waits(nc)
    return nc


def pack_shared(wq_w, wq_b, wk_w, wk_b, wv_w, wv_b,
                lambda_q1, lambda_k1, lambda_q2, lambda_k2):
    lam = (np.exp(lambda_q1 * lambda_k1) - np.exp(lambda_q2 * lambda_k2)
           + np.float32(0.8)).astype(np.float32)
    sc = np.float32(SCALE)
    # M_h = Wq_h^T Wk_h * scale; device computes y_h = x @ M_h, then
    # scores_h = y_h @ x^T (+ bias row).  Packed like a projection weight
    # W' = M_h^T with [o-block, contract, c-tile, out] layout.
    mcat = np.concatenate(
        [(wq_w[:DH].T @ wk_w[:DH]).T * sc,
         (wq_w[DH:].T @ wk_w[DH:]).T * sc], axis=0)
    # j-dependent rank-1 bias term: scores_h[i,j] += x_j . (Wk_h^T bq_h) * sc
    ch = np.stack([wk_w[:DH].T @ wq_b[:DH] * sc,
                   wk_w[DH:].T @ wq_b[DH:] * sc])          # [2, DM]
    return {
        "wm": np.ascontiguousarray(
            mcat.reshape(NOB, P, NT, P).transpose(0, 3, 2, 1)).astype(ml_dtypes.bfloat16),
        "wve": np.ascontiguousarray(
            wv_w.reshape(NODT, 512, NT, P).transpose(0, 3, 2, 1)).astype(ml_dtypes.bfloat16),
        "lamb": np.ascontiguousarray(np.broadcast_to(lam[None, :], (P, S))).astype(ml_dtypes.bfloat16),
        "bvb": np.ascontiguousarray(np.broadcast_to(
            wv_b.reshape(1, NODT, 512), (P, NODT, 512))).astype(ml_dtypes.bfloat16),
        "ones": np.ones(P, np.float32),
    }, ch


def make_in_maps(x, wq_w, wq_b, wk_w, wk_b, wv_w, wv_b,
                 lambda_q1, lambda_k1, lambda_q2, lambda_k2):
    shared, ch = pack_shared(wq_w, wq_b, wk_w, wk_b, wv_w, wv_b,
                             lambda_q1, lambda_k1, lambda_q2, lambda_k2)
    maps = []
    for b in range(B):
        xb = x[b]                                          # [S, DM]
        xtb = xb.T.reshape(NT, P, S).transpose(1, 0, 2)     # [P, NT, S]
        xre = xb.reshape(NJB, P, DM).transpose(1, 0, 2)     # [P, NJB, DM]
        maps.append({**shared,
                     "xtb": np.ascontiguousarray(xtb).astype(ml_dtypes.bfloat16),
                     "xre": np.ascontiguousarray(xre).astype(ml_dtypes.bfloat16),
                     "whs": np.ascontiguousarray(xb @ ch.T).T.copy()})
    return maps


_NC_CACHE = None


def get_nc():
    global _NC_CACHE
    if _NC_CACHE is None:
        _NC_CACHE = build_nc()
    return _NC_CACHE


def kernel(x, wq_w, wq_b, wk_w, wk_b, wv_w, wv_b,
           lambda_q1, lambda_k1, lambda_q2, lambda_k2):
    args = [np.asarray(a, dtype=np.float32) for a in
            (x, wq_w, wq_b, wk_w, wk_b, wv_w, wv_b,
             lambda_q1, lambda_k1, lambda_q2, lambda_k2)]
    nc = get_nc()
    in_maps = make_in_maps(*args)
    res = run_bass_kernel_spmd(nc, in_maps, list(range(B)))
    return np.stack([res.results[b]["out"] for b in range(B)]).astype(np.float32)
